# revision 3
# baseline (speedup 1.0000x reference)
"""BFP (block floating point) quantizer for Trainium2, 8 NeuronCores.

Reference semantics (BITWIDTH=16, BLOCK_SIZE=16, AXIS=1):
  per 16-element block along axis 1:
    max_abs = max |x|                     (block reduction)
    shared_exp = frexp(max_abs).e - 1
    step = 2^(shared_exp - 6)
    q = clip(round_half_even(x / step), -127, 127) * step
    q = 0 where max_abs == 0

Kernel mapping (per [128, 8192] f32 tile, blocks of 16 on the free axis):
  1. m = tensor_reduce(max, abs) over [128, 512, 16]        -> block max-abs
  2. exponent bit tricks on the int32 view of m:
       masked     = m_bits & 0x7F800000          (sign is 0, m >= 0)
       step_bits  = max(masked, 7<<23) - 6<<23   (power of two, exact;
                                                  the max() guards all-zero /
                                                  denormal blocks)
       rstep_bits = 0x7F000000 - step_bits       (exact reciprocal: exponents
                                                  sum to 254)
  3. y_i8 = tensor_tensor(x, rstep_bcast, mult) with int8 output.
     x * rstep is exact in fp32 (power-of-two scale); the DVE output
     converter does RNE + saturation, which implements round-half-even and
     the upper clip in one pass (verified bit-exact on HW).  The only
     deviation from the reference is y in (-128, -127.5] -> -128 (reference
     clips to -127), fixed in step 4.
  4. q = scalar_tensor_tensor(y_i8, -127.0, step_bcast, max, mult) -> f32.

Sharding: trivially data-parallel on axis 0; each of the 8 cores gets a
[1024, 8192] row shard and runs 8 [128, 8192] tiles.
"""

import sys

for _p in ("/opt/trn_rl_repo",):
    if _p not in sys.path:
        sys.path.append(_p)

import json

import numpy as np

N_CORES = 8
R_FULL = 8192
C = 8192
R_LOCAL = R_FULL // N_CORES  # 1024
P = 128
BLK = 16
NB = C // BLK  # 512
N_TILES = R_LOCAL // P  # 8


# ---------------------------------------------------------------------------
# Workaround for this container's walrus build: it encodes at most ONE
# semaphore wait per instruction ("Too many sync wait commands").  Rewrite the
# serialized BIR so any instruction with N>1 waits is preceded by N-1
# same-engine NoOps carrying one wait each.
# ---------------------------------------------------------------------------
def _split_multiwaits(bir_json: bytes) -> bytes:
    j = json.loads(bir_json)
    ctr = 0
    changed = False
    for fn in j.get("functions", []):
        for bb in fn.get("blocks", []):
            new_insts = []
            for ins in bb.get("instructions", []):
                si = ins.get("sync_info")
                waits = (si or {}).get("on_wait") or []
                if len(waits) > 1:
                    changed = True
                    for w in waits[:-1]:
                        ctr += 1
                        carrier = {
                            "engine": ins["engine"],
                            "ins": [],
                            "outs": [],
                            "name": f"WSPLIT-{ctr}",
                            "opcode": "NoOp",
                            "text_hint": "wait_split",
                            "sync_info": {"on_wait": [w], "on_update": []},
                        }
                        if "debug" in ins:
                            carrier["debug"] = ins["debug"]
                        new_insts.append(carrier)
                    si["on_wait"] = [waits[-1]]
                new_insts.append(ins)
            bb["instructions"] = new_insts
    if not changed:
        return bir_json
    return json.dumps(j).encode()


_hook_applied = False


def _apply_bir_fix():
    global _hook_applied
    if _hook_applied:
        return
    _hook_applied = True
    from concourse import bass2jax

    orig = bass2jax.compile_bir_kernel

    def wrapper(bir_json, tmpdir, neff_name="file.neff"):
        return orig(_split_multiwaits(bytes(bir_json)), tmpdir, neff_name)

    bass2jax.compile_bir_kernel = wrapper


# ---------------------------------------------------------------------------
# Program construction
# ---------------------------------------------------------------------------
def build_program(reps: int = 1):
    """reps>1 wraps the whole tile loop in a dynamic For_i — used only for
    benchmarking (amortizes the ~80ms axon dispatch overhead)."""
    from contextlib import nullcontext

    import concourse.bass as bass
    import concourse.tile as tile
    from concourse import mybir

    F32 = mybir.dt.float32
    I32 = mybir.dt.int32
    I8 = mybir.dt.int8

    nc = bass.Bass("TRN2", target_bir_lowering=False)
    x_ext = nc.dram_tensor("x", [R_LOCAL, C], F32, kind="ExternalInput")
    out_ext = nc.dram_tensor("out", [R_LOCAL, C], F32, kind="ExternalOutput")

    with tile.TileContext(nc) as tc:
        with (
            tc.tile_pool(name="xin", bufs=2) as xin,
            tc.tile_pool(name="qout", bufs=2) as qout,
            tc.tile_pool(name="i8p", bufs=2) as i8p,
            tc.tile_pool(name="small", bufs=2) as small,
            tc.tile_pool(name="consts", bufs=1) as consts,
            tc.For_i(0, reps, 1) if reps > 1 else nullcontext(),
        ):
            # step_bits + rstep_bits = 254 << 23
            csum = consts.tile([P, 1], I32)
            nc.vector.memset(csum, 0x7F000000)

            for i in range(N_TILES):
                rows = slice(i * P, (i + 1) * P)
                # chunk the first tile's load and the last tile's store so the
                # pipeline ramp/tail is ~3us instead of ~12us
                in_chunks = 4 if i == 0 else 1
                out_chunks = 4 if i == N_TILES - 1 else 1
                chunks = max(in_chunks, out_chunks)

                x_t = xin.tile([P, C], F32)
                for c in range(in_chunks):
                    cs = slice(c * (C // in_chunks), (c + 1) * (C // in_chunks))
                    nc.sync.dma_start(out=x_t[:, cs], in_=x_ext[rows, cs])
                x3 = x_t.rearrange("p (b k) -> p b k", k=BLK)

                m = small.tile([P, NB], F32, tag="m")
                step = small.tile([P, NB], F32, tag="step")
                rstep = small.tile([P, NB], F32, tag="rstep")
                y8 = i8p.tile([P, NB, BLK], I8)
                q = qout.tile([P, C], F32)
                q3 = q.rearrange("p (b k) -> p b k", k=BLK)

                nbc = NB // chunks
                for c in range(chunks):
                    bs = slice(c * nbc, (c + 1) * nbc)
                    nc.vector.tensor_reduce(
                        out=m[:, bs],
                        in_=x3[:, bs, :],
                        axis=mybir.AxisListType.X,
                        op=mybir.AluOpType.max,
                        apply_absolute_value=True,
                    )
                    nc.vector.tensor_scalar(
                        out=step[:, bs].bitcast(I32),
                        in0=m[:, bs].bitcast(I32),
                        scalar1=0x7F800000,
                        scalar2=None,
                        op0=mybir.AluOpType.bitwise_and,
                    )
                    nc.vector.tensor_scalar(
                        out=step[:, bs].bitcast(I32),
                        in0=step[:, bs].bitcast(I32),
                        scalar1=0x03800000,
                        scalar2=0x03000000,
                        op0=mybir.AluOpType.max,
                        op1=mybir.AluOpType.subtract,
                    )
                    nc.vector.scalar_tensor_tensor(
                        out=rstep[:, bs].bitcast(I32),
                        in0=csum.bitcast(I32).broadcast_to((P, nbc)),
                        scalar=0,
                        in1=step[:, bs].bitcast(I32),
                        op0=mybir.AluOpType.bypass,
                        op1=mybir.AluOpType.subtract,
                    )
                    nc.vector.tensor_tensor(
                        out=y8[:, bs, :],
                        in0=x3[:, bs, :],
                        in1=rstep[:, bs].unsqueeze(2).broadcast_to((P, nbc, BLK)),
                        op=mybir.AluOpType.mult,
                    )
                    nc.vector.scalar_tensor_tensor(
                        out=q3[:, bs, :],
                        in0=y8[:, bs, :],
                        scalar=-127.0,
                        in1=step[:, bs].unsqueeze(2).broadcast_to((P, nbc, BLK)),
                        op0=mybir.AluOpType.max,
                        op1=mybir.AluOpType.mult,
                    )

                for c in range(out_chunks):
                    cs = slice(c * (C // out_chunks), (c + 1) * (C // out_chunks))
                    nc.scalar.dma_start(out=out_ext[rows, cs], in_=q[:, cs])
    return nc


_cached_nc = None


def run(x: np.ndarray, trace: bool = False):
    """Run the SPMD kernel on 8 cores; returns (full_output, BassKernelResults)."""
    global _cached_nc
    _apply_bir_fix()
    from concourse.bass_utils import run_bass_kernel_spmd

    assert x.shape == (R_FULL, C) and x.dtype == np.float32
    if _cached_nc is None:
        _cached_nc = build_program()

    in_maps = [
        {"x": np.ascontiguousarray(x[i * R_LOCAL : (i + 1) * R_LOCAL])}
        for i in range(N_CORES)
    ]
    res = run_bass_kernel_spmd(
        _cached_nc, in_maps, list(range(N_CORES)), trace=trace
    )
    out = np.concatenate([r["out"] for r in res.results], axis=0)
    return out, res


def kernel(x: np.ndarray) -> np.ndarray:
    out, _ = run(x, trace=False)
    return out


# revision 4
# speedup vs baseline: 1.0223x; 1.0223x over previous
"""BFP (block floating point) quantizer for Trainium2, 8 NeuronCores.

Reference semantics (BITWIDTH=16, BLOCK_SIZE=16, AXIS=1):
  per 16-element block along axis 1:
    max_abs = max |x|                     (block reduction)
    shared_exp = frexp(max_abs).e - 1
    step = 2^(shared_exp - 6)
    q = clip(round_half_even(x / step), -127, 127) * step
    q = 0 where max_abs == 0

Kernel mapping (per [128, 8192] f32 tile, blocks of 16 on the free axis):
  1. m = tensor_reduce(max, abs) over [128, 512, 16]        -> block max-abs
  2. exponent bit tricks on the int32 view of m:
       masked     = m_bits & 0x7F800000          (sign is 0, m >= 0)
       step_bits  = max(masked, 7<<23) - 6<<23   (power of two, exact;
                                                  the max() guards all-zero /
                                                  denormal blocks)
       rstep_bits = 0x7F000000 - step_bits       (exact reciprocal: exponents
                                                  sum to 254)
  3. y_i8 = tensor_tensor(x, rstep_bcast, mult) with int8 output.
     x * rstep is exact in fp32 (power-of-two scale); the DVE output
     converter does RNE + saturation, which implements round-half-even and
     the upper clip in one pass (verified bit-exact on HW).  The only
     deviation from the reference is y in (-128, -127.5] -> -128 (reference
     clips to -127), fixed in step 4.
  4. q = scalar_tensor_tensor(y_i8, -127.0, step_bcast, max, mult) -> f32.

Sharding: trivially data-parallel on axis 0; each of the 8 cores gets a
[1024, 8192] row shard and runs 8 [128, 8192] tiles.
"""

import sys

for _p in ("/opt/trn_rl_repo",):
    if _p not in sys.path:
        sys.path.append(_p)

import json

import numpy as np

N_CORES = 8
R_FULL = 8192
C = 8192
R_LOCAL = R_FULL // N_CORES  # 1024
P = 128
BLK = 16
NB = C // BLK  # 512
N_TILES = R_LOCAL // P  # 8


# ---------------------------------------------------------------------------
# Workaround for this container's walrus build: it encodes at most ONE
# semaphore wait per instruction ("Too many sync wait commands").  Rewrite the
# serialized BIR so any instruction with N>1 waits is preceded by N-1
# same-engine NoOps carrying one wait each.
# ---------------------------------------------------------------------------
def _split_multiwaits(bir_json: bytes) -> bytes:
    j = json.loads(bir_json)
    ctr = 0
    changed = False
    for fn in j.get("functions", []):
        for bb in fn.get("blocks", []):
            new_insts = []
            for ins in bb.get("instructions", []):
                si = ins.get("sync_info")
                waits = (si or {}).get("on_wait") or []
                if len(waits) > 1:
                    changed = True
                    for w in waits[:-1]:
                        ctr += 1
                        carrier = {
                            "engine": ins["engine"],
                            "ins": [],
                            "outs": [],
                            "name": f"WSPLIT-{ctr}",
                            "opcode": "NoOp",
                            "text_hint": "wait_split",
                            "sync_info": {"on_wait": [w], "on_update": []},
                        }
                        if "debug" in ins:
                            carrier["debug"] = ins["debug"]
                        new_insts.append(carrier)
                    si["on_wait"] = [waits[-1]]
                new_insts.append(ins)
            bb["instructions"] = new_insts
    if not changed:
        return bir_json
    return json.dumps(j).encode()


_hook_applied = False


def _apply_bir_fix():
    global _hook_applied
    if _hook_applied:
        return
    _hook_applied = True
    from concourse import bass2jax

    orig = bass2jax.compile_bir_kernel

    def wrapper(bir_json, tmpdir, neff_name="file.neff"):
        return orig(_split_multiwaits(bytes(bir_json)), tmpdir, neff_name)

    bass2jax.compile_bir_kernel = wrapper


# ---------------------------------------------------------------------------
# Program construction
# ---------------------------------------------------------------------------
def build_program(reps: int = 1):
    """reps>1 wraps the whole tile loop in a dynamic For_i — used only for
    benchmarking (amortizes the ~80ms axon dispatch overhead)."""
    from contextlib import nullcontext

    import concourse.bass as bass
    import concourse.tile as tile
    from concourse import mybir

    F32 = mybir.dt.float32
    I32 = mybir.dt.int32
    I8 = mybir.dt.int8

    nc = bass.Bass("TRN2", target_bir_lowering=False)
    x_ext = nc.dram_tensor("x", [R_LOCAL, C], F32, kind="ExternalInput")
    out_ext = nc.dram_tensor("out", [R_LOCAL, C], F32, kind="ExternalOutput")

    with tile.TileContext(nc) as tc:
        with (
            tc.tile_pool(name="xin", bufs=2) as xin,
            tc.tile_pool(name="qout", bufs=2) as qout,
            tc.tile_pool(name="i8p", bufs=2) as i8p,
            tc.tile_pool(name="small", bufs=2) as small,
            tc.tile_pool(name="consts", bufs=1) as consts,
            tc.For_i(0, reps, 1) if reps > 1 else nullcontext(),
        ):
            # step_bits + rstep_bits = 254 << 23
            csum = consts.tile([P, 1], I32)
            nc.vector.memset(csum, 0x7F000000)

            for i in range(N_TILES):
                rows = slice(i * P, (i + 1) * P)

                x_t = xin.tile([P, C], F32)
                nc.sync.dma_start(out=x_t, in_=x_ext[rows, :])
                x3 = x_t.rearrange("p (b k) -> p b k", k=BLK)

                m = small.tile([P, NB], F32, tag="m")
                nc.vector.tensor_reduce(
                    out=m,
                    in_=x3,
                    axis=mybir.AxisListType.X,
                    op=mybir.AluOpType.max,
                    apply_absolute_value=True,
                )

                step = small.tile([P, NB], F32, tag="step")
                nc.vector.tensor_scalar(
                    out=step.bitcast(I32),
                    in0=m.bitcast(I32),
                    scalar1=0x7F800000,
                    scalar2=None,
                    op0=mybir.AluOpType.bitwise_and,
                )
                nc.vector.tensor_scalar(
                    out=step.bitcast(I32),
                    in0=step.bitcast(I32),
                    scalar1=0x03800000,
                    scalar2=0x03000000,
                    op0=mybir.AluOpType.max,
                    op1=mybir.AluOpType.subtract,
                )
                rstep = small.tile([P, NB], F32, tag="rstep")
                nc.vector.scalar_tensor_tensor(
                    out=rstep.bitcast(I32),
                    in0=csum.bitcast(I32).broadcast_to((P, NB)),
                    scalar=0,
                    in1=step.bitcast(I32),
                    op0=mybir.AluOpType.bypass,
                    op1=mybir.AluOpType.subtract,
                )

                y8 = i8p.tile([P, NB, BLK], I8)
                nc.vector.tensor_tensor(
                    out=y8,
                    in0=x3,
                    in1=rstep.unsqueeze(2).broadcast_to((P, NB, BLK)),
                    op=mybir.AluOpType.mult,
                )

                q = qout.tile([P, C], F32)
                q3 = q.rearrange("p (b k) -> p b k", k=BLK)
                nc.vector.scalar_tensor_tensor(
                    out=q3,
                    in0=y8,
                    scalar=-127.0,
                    in1=step.unsqueeze(2).broadcast_to((P, NB, BLK)),
                    op0=mybir.AluOpType.max,
                    op1=mybir.AluOpType.mult,
                )

                nc.scalar.dma_start(out=out_ext[rows, :], in_=q)
    return nc


_cached_nc = None


def run(x: np.ndarray, trace: bool = False):
    """Run the SPMD kernel on 8 cores; returns (full_output, BassKernelResults)."""
    global _cached_nc
    _apply_bir_fix()
    from concourse.bass_utils import run_bass_kernel_spmd

    assert x.shape == (R_FULL, C) and x.dtype == np.float32
    if _cached_nc is None:
        _cached_nc = build_program()

    in_maps = [
        {"x": np.ascontiguousarray(x[i * R_LOCAL : (i + 1) * R_LOCAL])}
        for i in range(N_CORES)
    ]
    res = run_bass_kernel_spmd(
        _cached_nc, in_maps, list(range(N_CORES)), trace=trace
    )
    out = np.concatenate([r["out"] for r in res.results], axis=0)
    return out, res


def kernel(x: np.ndarray) -> np.ndarray:
    out, _ = run(x, trace=False)
    return out


# revision 6
# speedup vs baseline: 2.8306x; 2.7690x over previous
"""BFP (block floating point) quantizer for Trainium2, 8 NeuronCores.

Reference semantics (BITWIDTH=16, BLOCK_SIZE=16, AXIS=1):
  per 16-element block along axis 1:
    max_abs = max |x|                     (block reduction)
    shared_exp = frexp(max_abs).e - 1
    step = 2^(shared_exp - 6)
    q = clip(round_half_even(x / step), -127, 127) * step
    q = 0 where max_abs == 0

Kernel mapping (per [128, 8192] f32 tile, blocks of 16 on the free axis):
  1. m = tensor_reduce(max, abs) over [128, 512, 16]        -> block max-abs
  2. exponent bit tricks on the int32 view of m:
       masked     = m_bits & 0x7F800000          (sign is 0, m >= 0)
       step_bits  = max(masked, 7<<23) - 6<<23   (power of two, exact;
                                                  the max() guards all-zero /
                                                  denormal blocks)
       rstep_bits = 0x7F000000 - step_bits       (exact reciprocal: exponents
                                                  sum to 254)
  3. y_i8 = tensor_tensor(x, rstep_bcast, mult) with int8 output.
     x * rstep is exact in fp32 (power-of-two scale); the DVE output
     converter does RNE + saturation, which implements round-half-even and
     the upper clip in one pass (verified bit-exact on HW).  The only
     deviation from the reference is y in (-128, -127.5] -> -128 (reference
     clips to -127), fixed in step 4.
  4. q = scalar_tensor_tensor(y_i8, -127.0, step_bcast, max, mult) -> f32.

Sharding: trivially data-parallel on axis 0; each of the 8 cores gets a
[1024, 8192] row shard and runs 8 [128, 8192] tiles.
"""

import sys

for _p in ("/opt/trn_rl_repo",):
    if _p not in sys.path:
        sys.path.append(_p)

import json

import numpy as np

N_CORES = 8
R_FULL = 8192
C = 8192
R_LOCAL = R_FULL // N_CORES  # 1024
P = 128
BLK = 16
NB = C // BLK  # 512
N_TILES = R_LOCAL // P  # 8


# ---------------------------------------------------------------------------
# Workaround for this container's walrus build: it encodes at most ONE
# semaphore wait per instruction ("Too many sync wait commands").  Rewrite the
# serialized BIR so any instruction with N>1 waits is preceded by N-1
# same-engine NoOps carrying one wait each.
# ---------------------------------------------------------------------------
def _split_multiwaits(bir_json: bytes) -> bytes:
    j = json.loads(bir_json)
    ctr = 0
    changed = False
    for fn in j.get("functions", []):
        for bb in fn.get("blocks", []):
            new_insts = []
            for ins in bb.get("instructions", []):
                si = ins.get("sync_info")
                waits = (si or {}).get("on_wait") or []
                if len(waits) > 1:
                    changed = True
                    for w in waits[:-1]:
                        ctr += 1
                        carrier = {
                            "engine": ins["engine"],
                            "ins": [],
                            "outs": [],
                            "name": f"WSPLIT-{ctr}",
                            "opcode": "NoOp",
                            "text_hint": "wait_split",
                            "sync_info": {"on_wait": [w], "on_update": []},
                        }
                        if "debug" in ins:
                            carrier["debug"] = ins["debug"]
                        new_insts.append(carrier)
                    si["on_wait"] = [waits[-1]]
                new_insts.append(ins)
            bb["instructions"] = new_insts
    if not changed:
        return bir_json
    return json.dumps(j).encode()


_hook_applied = False


def _apply_bir_fix():
    global _hook_applied
    if _hook_applied:
        return
    _hook_applied = True
    from concourse import bass2jax

    orig = bass2jax.compile_bir_kernel

    def wrapper(bir_json, tmpdir, neff_name="file.neff"):
        return orig(_split_multiwaits(bytes(bir_json)), tmpdir, neff_name)

    bass2jax.compile_bir_kernel = wrapper


# ---------------------------------------------------------------------------
# Program construction
# ---------------------------------------------------------------------------
def build_program(reps: int = 1):
    """reps>1 wraps the whole tile loop in a dynamic For_i — used only for
    benchmarking (amortizes the ~80ms axon dispatch overhead)."""
    from contextlib import nullcontext

    import concourse.bass as bass
    import concourse.tile as tile
    from concourse import mybir

    F32 = mybir.dt.float32
    I32 = mybir.dt.int32
    I8 = mybir.dt.int8

    nc = bass.Bass("TRN2", target_bir_lowering=False)
    x_ext = nc.dram_tensor("x", [R_LOCAL, C], F32, kind="ExternalInput")
    out_ext = nc.dram_tensor("out", [R_LOCAL, C], F32, kind="ExternalOutput")

    with tile.TileContext(nc) as tc:
        with (
            tc.tile_pool(name="xin", bufs=2) as xin,
            tc.tile_pool(name="qout", bufs=2) as qout,
            tc.tile_pool(name="i8p", bufs=2) as i8p,
            tc.tile_pool(name="small", bufs=2) as small,
            tc.tile_pool(name="consts", bufs=1) as consts,
            tc.For_i(0, reps, 1) if reps > 1 else nullcontext(),
        ):
            # step_bits + rstep_bits = 254 << 23
            csum = consts.tile([P, 1], I32)
            nc.vector.memset(csum, 0x7F000000)

            for i in range(N_TILES):
                rows = slice(i * P, (i + 1) * P)

                x_t = xin.tile([P, C], F32)
                x3 = x_t.rearrange("p (b k) -> p b k", k=BLK)
                m = small.tile([P, NB], F32, tag="m")
                # chunk the first tile's load+reduce so DVE starts at ~3us
                # instead of ~13us (pipeline ramp)
                n_in = 4 if i == 0 else 1
                cw, bw = C // n_in, NB // n_in
                for c in range(n_in):
                    nc.sync.dma_start(
                        out=x_t[:, c * cw : (c + 1) * cw],
                        in_=x_ext[rows, c * cw : (c + 1) * cw],
                    )
                    nc.vector.tensor_reduce(
                        out=m[:, c * bw : (c + 1) * bw],
                        in_=x3[:, c * bw : (c + 1) * bw, :],
                        axis=mybir.AxisListType.X,
                        op=mybir.AluOpType.max,
                        apply_absolute_value=True,
                    )

                step = small.tile([P, NB], F32, tag="step")
                nc.vector.tensor_scalar(
                    out=step.bitcast(I32),
                    in0=m.bitcast(I32),
                    scalar1=0x7F800000,
                    scalar2=None,
                    op0=mybir.AluOpType.bitwise_and,
                )
                nc.vector.tensor_scalar(
                    out=step.bitcast(I32),
                    in0=step.bitcast(I32),
                    scalar1=0x03800000,
                    scalar2=0x03000000,
                    op0=mybir.AluOpType.max,
                    op1=mybir.AluOpType.subtract,
                )
                rstep = small.tile([P, NB], F32, tag="rstep")
                nc.vector.scalar_tensor_tensor(
                    out=rstep.bitcast(I32),
                    in0=csum.bitcast(I32).broadcast_to((P, NB)),
                    scalar=0,
                    in1=step.bitcast(I32),
                    op0=mybir.AluOpType.bypass,
                    op1=mybir.AluOpType.subtract,
                )

                y8 = i8p.tile([P, NB, BLK], I8)
                nc.vector.tensor_tensor(
                    out=y8,
                    in0=x3,
                    in1=rstep.unsqueeze(2).broadcast_to((P, NB, BLK)),
                    op=mybir.AluOpType.mult,
                )

                q = qout.tile([P, C], F32)
                q3 = q.rearrange("p (b k) -> p b k", k=BLK)
                # chunk the last tile's scale+store so the trailing store is
                # ~3us instead of ~15us (pipeline tail)
                n_out = 4 if i == N_TILES - 1 else 1
                cw, bw = C // n_out, NB // n_out
                for c in range(n_out):
                    bs = slice(c * bw, (c + 1) * bw)
                    nc.vector.scalar_tensor_tensor(
                        out=q3[:, bs, :],
                        in0=y8[:, bs, :],
                        scalar=-127.0,
                        in1=step[:, bs].unsqueeze(2).broadcast_to((P, bw, BLK)),
                        op0=mybir.AluOpType.max,
                        op1=mybir.AluOpType.mult,
                    )
                    nc.scalar.dma_start(
                        out=out_ext[rows, c * cw : (c + 1) * cw],
                        in_=q[:, c * cw : (c + 1) * cw],
                    )
    return nc


_cached_nc = None


def run(x: np.ndarray, trace: bool = False):
    """Run the SPMD kernel on 8 cores; returns (full_output, BassKernelResults)."""
    global _cached_nc
    _apply_bir_fix()
    from concourse.bass_utils import run_bass_kernel_spmd

    assert x.shape == (R_FULL, C) and x.dtype == np.float32
    if _cached_nc is None:
        _cached_nc = build_program()

    in_maps = [
        {"x": np.ascontiguousarray(x[i * R_LOCAL : (i + 1) * R_LOCAL])}
        for i in range(N_CORES)
    ]
    res = run_bass_kernel_spmd(
        _cached_nc, in_maps, list(range(N_CORES)), trace=trace
    )
    out = np.concatenate([r["out"] for r in res.results], axis=0)
    return out, res


def kernel(x: np.ndarray) -> np.ndarray:
    out, _ = run(x, trace=False)
    return out


# revision 9
# speedup vs baseline: 2.8401x; 1.0034x over previous
"""BFP (block floating point) quantizer for Trainium2, 8 NeuronCores.

Reference semantics (BITWIDTH=16, BLOCK_SIZE=16, AXIS=1):
  per 16-element block along axis 1:
    max_abs = max |x|                     (block reduction)
    shared_exp = frexp(max_abs).e - 1
    step = 2^(shared_exp - 6)
    q = clip(round_half_even(x / step), -127, 127) * step
    q = 0 where max_abs == 0

Kernel mapping (per [128, 8192] f32 tile, blocks of 16 on the free axis):
  1. m = tensor_reduce(max, abs) over [128, 512, 16]        -> block max-abs
  2. exponent bit tricks on the int32 view of m:
       masked     = m_bits & 0x7F800000          (sign is 0, m >= 0)
       step_bits  = max(masked, 7<<23) - 6<<23   (power of two, exact;
                                                  the max() guards all-zero /
                                                  denormal blocks)
       rstep_bits = 0x7F000000 - step_bits       (exact reciprocal: exponents
                                                  sum to 254)
  3. y_i8 = tensor_tensor(x, rstep_bcast, mult) with int8 output.
     x * rstep is exact in fp32 (power-of-two scale); the DVE output
     converter does RNE + saturation, which implements round-half-even and
     the upper clip in one pass (verified bit-exact on HW).  The only
     deviation from the reference is y in (-128, -127.5] -> -128 (reference
     clips to -127), fixed in step 4.
  4. q = scalar_tensor_tensor(y_i8, -127.0, step_bcast, max, mult) -> f32.

Sharding: trivially data-parallel on axis 0; each of the 8 cores gets a
[1024, 8192] row shard and runs 8 [128, 8192] tiles.
"""

import sys

for _p in ("/opt/trn_rl_repo",):
    if _p not in sys.path:
        sys.path.append(_p)

import json

import numpy as np

N_CORES = 8
R_FULL = 8192
C = 8192
R_LOCAL = R_FULL // N_CORES  # 1024
P = 128
BLK = 16
NB = C // BLK  # 512
N_TILES = R_LOCAL // P  # 8


# ---------------------------------------------------------------------------
# Workaround for this container's walrus build: it encodes at most ONE
# semaphore wait per instruction ("Too many sync wait commands").  Rewrite the
# serialized BIR so any instruction with N>1 waits is preceded by N-1
# same-engine NoOps carrying one wait each.
# ---------------------------------------------------------------------------
def _split_multiwaits(bir_json: bytes) -> bytes:
    j = json.loads(bir_json)
    ctr = 0
    changed = False
    for fn in j.get("functions", []):
        for bb in fn.get("blocks", []):
            new_insts = []
            for ins in bb.get("instructions", []):
                si = ins.get("sync_info")
                waits = (si or {}).get("on_wait") or []
                if len(waits) > 1:
                    changed = True
                    for w in waits[:-1]:
                        ctr += 1
                        carrier = {
                            "engine": ins["engine"],
                            "ins": [],
                            "outs": [],
                            "name": f"WSPLIT-{ctr}",
                            "opcode": "NoOp",
                            "text_hint": "wait_split",
                            "sync_info": {"on_wait": [w], "on_update": []},
                        }
                        if "debug" in ins:
                            carrier["debug"] = ins["debug"]
                        new_insts.append(carrier)
                    si["on_wait"] = [waits[-1]]
                new_insts.append(ins)
            bb["instructions"] = new_insts
    if not changed:
        return bir_json
    return json.dumps(j).encode()


_hook_applied = False


def _apply_bir_fix():
    global _hook_applied
    if _hook_applied:
        return
    _hook_applied = True
    from concourse import bass2jax

    orig = bass2jax.compile_bir_kernel

    def wrapper(bir_json, tmpdir, neff_name="file.neff"):
        return orig(_split_multiwaits(bytes(bir_json)), tmpdir, neff_name)

    bass2jax.compile_bir_kernel = wrapper


# ---------------------------------------------------------------------------
# Program construction
# ---------------------------------------------------------------------------
def build_program(reps: int = 1):
    """reps>1 wraps the whole tile loop in a dynamic For_i — used only for
    benchmarking (amortizes the ~80ms axon dispatch overhead)."""
    from contextlib import nullcontext

    import concourse.bass as bass
    import concourse.tile as tile
    from concourse import mybir

    F32 = mybir.dt.float32
    I32 = mybir.dt.int32
    I8 = mybir.dt.int8

    nc = bass.Bass("TRN2", target_bir_lowering=False)
    x_ext = nc.dram_tensor("x", [R_LOCAL, C], F32, kind="ExternalInput")
    out_ext = nc.dram_tensor("out", [R_LOCAL, C], F32, kind="ExternalOutput")

    with tile.TileContext(nc) as tc:
        with (
            tc.tile_pool(name="xin", bufs=2) as xin,
            tc.tile_pool(name="qout", bufs=2) as qout,
            tc.tile_pool(name="i8p", bufs=2) as i8p,
            tc.tile_pool(name="small", bufs=2) as small,
            tc.tile_pool(name="consts", bufs=1) as consts,
            tc.For_i(0, reps, 1) if reps > 1 else nullcontext(),
        ):
            # step_bits + rstep_bits = 254 << 23
            csum = consts.tile([P, 1], I32)
            nc.vector.memset(csum, 0x7F000000)

            for i in range(N_TILES):
                rows = slice(i * P, (i + 1) * P)

                x_t = xin.tile([P, C], F32)
                x3 = x_t.rearrange("p (b k) -> p b k", k=BLK)
                m = small.tile([P, NB], F32, tag="m")
                # chunk the first tile's load+reduce (progressively sized) so
                # DVE starts as soon as the first small chunk lands
                in_widths = [2048, 2048, 2048, 2048] if i == 0 else [C]
                c0 = 0
                for cw in in_widths:
                    nc.sync.dma_start(
                        out=x_t[:, c0 : c0 + cw],
                        in_=x_ext[rows, c0 : c0 + cw],
                    )
                    nc.vector.tensor_reduce(
                        out=m[:, c0 // BLK : (c0 + cw) // BLK],
                        in_=x3[:, c0 // BLK : (c0 + cw) // BLK, :],
                        axis=mybir.AxisListType.X,
                        op=mybir.AluOpType.max,
                        apply_absolute_value=True,
                    )
                    c0 += cw

                step = small.tile([P, NB], F32, tag="step")
                nc.vector.tensor_scalar(
                    out=step.bitcast(I32),
                    in0=m.bitcast(I32),
                    scalar1=0x7F800000,
                    scalar2=None,
                    op0=mybir.AluOpType.bitwise_and,
                )
                nc.vector.tensor_scalar(
                    out=step.bitcast(I32),
                    in0=step.bitcast(I32),
                    scalar1=0x03800000,
                    scalar2=0x03000000,
                    op0=mybir.AluOpType.max,
                    op1=mybir.AluOpType.subtract,
                )
                rstep = small.tile([P, NB], F32, tag="rstep")
                nc.vector.scalar_tensor_tensor(
                    out=rstep.bitcast(I32),
                    in0=csum.bitcast(I32).broadcast_to((P, NB)),
                    scalar=0,
                    in1=step.bitcast(I32),
                    op0=mybir.AluOpType.bypass,
                    op1=mybir.AluOpType.subtract,
                )

                y8 = i8p.tile([P, NB, BLK], I8)
                nc.vector.tensor_tensor(
                    out=y8,
                    in0=x3,
                    in1=rstep.unsqueeze(2).broadcast_to((P, NB, BLK)),
                    op=mybir.AluOpType.mult,
                )

                q = qout.tile([P, C], F32)
                q3 = q.rearrange("p (b k) -> p b k", k=BLK)
                # chunk the last tile's scale+store (shrinking sizes, stores
                # alternating across both HWDGE rings) so the trailing store
                # after the final DVE op is one small chunk
                out_widths = [4096, 2048, 1536, 512] if i == N_TILES - 1 else [C]
                c0 = 0
                for ci, cw in enumerate(out_widths):
                    bs = slice(c0 // BLK, (c0 + cw) // BLK)
                    bw = cw // BLK
                    nc.vector.scalar_tensor_tensor(
                        out=q3[:, bs, :],
                        in0=y8[:, bs, :],
                        scalar=-127.0,
                        in1=step[:, bs].unsqueeze(2).broadcast_to((P, bw, BLK)),
                        op0=mybir.AluOpType.max,
                        op1=mybir.AluOpType.mult,
                    )
                    eng = nc.scalar if ci % 2 == 0 else nc.sync
                    eng.dma_start(
                        out=out_ext[rows, c0 : c0 + cw],
                        in_=q[:, c0 : c0 + cw],
                    )
                    c0 += cw
    return nc


_cached_nc = None


def run(x: np.ndarray, trace: bool = False):
    """Run the SPMD kernel on 8 cores; returns (full_output, BassKernelResults)."""
    global _cached_nc
    _apply_bir_fix()
    from concourse.bass_utils import run_bass_kernel_spmd

    assert x.shape == (R_FULL, C) and x.dtype == np.float32
    if _cached_nc is None:
        _cached_nc = build_program()

    in_maps = [
        {"x": np.ascontiguousarray(x[i * R_LOCAL : (i + 1) * R_LOCAL])}
        for i in range(N_CORES)
    ]
    res = run_bass_kernel_spmd(
        _cached_nc, in_maps, list(range(N_CORES)), trace=trace
    )
    out = np.concatenate([r["out"] for r in res.results], axis=0)
    return out, res


def kernel(x: np.ndarray) -> np.ndarray:
    out, _ = run(x, trace=False)
    return out
